# revision 1
# baseline (speedup 1.0000x reference)
"""Device kernels + host middle for nn_Entropy_Hist (3x3x3 window entropy
histogram + top-k channel gather) on 8 trn2 cores.

Phase 1 (device): per core 16 channel slabs -> per-voxel bin bytes + boundary
distance (f16) + global min/max via AllReduce.
Host middle: exact histogram fixup for near-boundary samples, entropy, top-k.
Phase 2 (device): gather selected channel slabs.
"""

import numpy as np

import concourse.bass as bass
import concourse.bacc as bacc
import concourse.mybir as mybir
import concourse.tile as tile
from concourse.bass_utils import run_bass_kernel_spmd

N_CORES = 8
B, C, H, W, Z = 2, 64, 64, 64, 64
HP = H - 2          # 62 valid per spatial dim
P_SLAB = HP * HP * HP   # 238328 voxels per slab
SLABS_PER_CORE = (B * C) // N_CORES  # 16
PAIRS = SLABS_PER_CORE // 2          # 8
K26 = np.float32(1.0) / np.float32(26.0)  # folded into band weights
C100 = np.float32(100.0) - np.float32(K26)
BINS = 256
DENOM = (H + 2) * (W + 2) * (Z + 2)
FLT_MAX = np.float32(3.4e38)

# number of ij pair-tiles kept resident in SBUF (rest spill to DRAM scratch)
RESIDENT_PAIRS = 3


def build_band():
    """[128,128] f32: col m sums rows m-1..m+1 (within each 64 block), scaled
    by 1/26. Cols 0,63,64,127 are unused (garbage outputs)."""
    band = np.zeros((128, 128), np.float32)
    for blk in (0, 64):
        for m in range(1, 63):
            for k in (m - 1, m, m + 1):
                band[blk + k, blk + m] = K26
    return band


def build_phase1():
    nc = bacc.Bacc("TRN2", target_bir_lowering=False, debug=False,
                   num_devices=N_CORES)
    f32, f32r = mybir.dt.float32, mybir.dt.float32r
    imgp = nc.dram_tensor("imgp", [SLABS_PER_CORE, H, W, Z], f32r,
                          kind="ExternalInput")
    bandw = nc.dram_tensor("bandw", [128, 128], f32r, kind="ExternalInput")
    bins_o = nc.dram_tensor("bins", [SLABS_PER_CORE, HP * HP * HP],
                            mybir.dt.uint8, kind="ExternalOutput")
    d16_o = nc.dram_tensor("d16", [SLABS_PER_CORE, HP * HP * HP],
                           mybir.dt.float16, kind="ExternalOutput")
    mm_o = nc.dram_tensor("minmax", [1, 2], f32, kind="ExternalOutput")

    FD = HP * HP            # 3844 free elems per partition (h', z')
    # h' chunking for PSUM banks: chunks of 8 h' rows (<=512 free each)
    H_CHUNKS = [(i, min(8, HP - i)) for i in range(0, HP, 8)]

    with tile.TileContext(nc) as tc:
        with (
            tc.tile_pool(name="pool", bufs=1) as pool,
            tc.tile_pool(name="pdbuf", bufs=2) as pdbuf,
            tc.tile_pool(name="psum", bufs=2, space="PSUM") as psum,
            tc.tile_pool(name="dram", bufs=1, space="DRAM") as dram,
        )        :
            band_t = pool.tile([128, 128], f32r, tag="band")
            nc.sync.dma_start(band_t[:], bandw[:])

            # running per-partition max(ij) and min(ij)
            rx = pool.tile([128, 1], f32, tag="rx")
            rm = pool.tile([128, 1], f32, tag="rm")
            nc.vector.memset(rx[:], -FLT_MAX)
            nc.vector.memset(rm[:], FLT_MAX)

            ij_tiles = []
            ij_spill = []
            for p in range(PAIRS):
                # ---- load pair: partition = w (64 per slab), free = (h, z)
                tld = pdbuf.tile([128, H * Z], f32r, tag="tld")
                tld3 = tld[:].rearrange("p (h z) -> p h z", h=H)
                for half in range(2):
                    s = 2 * p + half
                    src = imgp[s].rearrange("h w z -> w h z")
                    nc.sync.dma_start(tld3[64 * half:64 * half + 64], src)

                # ---- a2 = (100 - k26) * center ; center = tld[w, h'+1, z'+1]
                a2 = pdbuf.tile([128, FD], f32, tag="a2")
                cen = tld3[:, 1:1 + HP, 1:1 + HP]
                nc.scalar.activation(a2[:], cen,
                                     mybir.ActivationFunctionType.Copy,
                                     scale=float(C100))

                # ---- PE: 9-shift band matmul -> psum = k26 * sum27
                # ij chunk-add pipelined behind each PSUM evacuation
                a1 = pdbuf.tile([128, FD], f32, tag="a1")
                if p < RESIDENT_PAIRS:
                    ij = pool.tile([128, FD], f32, tag=f"ij{p}")
                else:
                    ij = pdbuf.tile([128, FD], f32, tag="ij_sp")
                for (h0, hn) in H_CHUNKS:
                    ps = psum.tile([128, 8 * HP], f32, tag="ps")
                    out_ap = ps[:, 0:hn * HP]
                    n9 = 0
                    for dh in range(3):
                        for dk in range(3):
                            rhs = tld3[:, h0 + dh:h0 + dh + hn, dk:dk + HP]
                            nc.tensor.matmul(out_ap, band_t[:], rhs,
                                             start=(n9 == 0), stop=(n9 == 8))
                            n9 += 1
                    sl = slice(h0 * HP, (h0 + hn) * HP)
                    nc.scalar.activation(
                        a1[:, sl], out_ap,
                        mybir.ActivationFunctionType.Copy, scale=1.0)
                    nc.gpsimd.tensor_tensor(ij[:, sl], a1[:, sl], a2[:, sl],
                                            mybir.AluOpType.add)

                # patch garbage partitions 0,63,64,127 with valid neighbours
                # so full-partition reduces stay inside the true value range
                nc.sync.dma_start(ij[0:1, :], ij[1:2, :])
                nc.sync.dma_start(ij[63:64, :], ij[62:63, :])
                nc.sync.dma_start(ij[64:65, :], ij[65:66, :])
                nc.sync.dma_start(ij[127:128, :], ij[126:127, :])

                # ---- running min/max over valid rows
                pr = pool.tile([128, 2], f32, tag="pr")
                nc.vector.tensor_reduce(pr[:, 0:1], ij[:, :],
                                        mybir.AxisListType.XYZW,
                                        mybir.AluOpType.max)
                nc.vector.tensor_reduce(pr[:, 1:2], ij[:, :],
                                        mybir.AxisListType.XYZW,
                                        mybir.AluOpType.min)
                nc.vector.tensor_tensor(rx[:, :], rx[:, :],
                                        pr[:, 0:1], mybir.AluOpType.max)
                nc.vector.tensor_tensor(rm[:, :], rm[:, :],
                                        pr[:, 1:2], mybir.AluOpType.min)

                if p < RESIDENT_PAIRS:
                    ij_tiles.append(ij)
                    ij_spill.append(None)
                else:
                    sp = dram.tile([128, FD], f32, tag=f"sp{p}")
                    nc.sync.dma_start(sp[:], ij[:])
                    ij_tiles.append(None)
                    ij_spill.append(sp)

            # ---- global min/max: [max, -min] allreduce(max) then partition AR
            cin_s = pool.tile([128, 2], f32, tag="cin")
            nc.vector.tensor_copy(cin_s[:, 0:1], rx[:])
            nc.vector.tensor_scalar_mul(cin_s[:, 1:2], rm[:], -1.0)
            cin = dram.tile([128, 2], f32, tag="cc_in")
            cout = dram.tile([128, 2], f32, tag="cc_out", addr_space="Shared")
            nc.sync.dma_start(cin[:], cin_s[:])
            nc.gpsimd.collective_compute(
                "AllReduce", mybir.AluOpType.max,
                replica_groups=[list(range(N_CORES))],
                ins=[cin[:].opt()], outs=[cout[:].opt()],
            )
            car = pool.tile([128, 2], f32, tag="car")
            nc.sync.dma_start(car[:], cout[:])
            gmm = pool.tile([128, 2], f32, tag="gmm")
            import concourse.bass_isa as bass_isa
            nc.gpsimd.partition_all_reduce(gmm[:], car[:], 128,
                                           bass_isa.ReduceOp.max)
            nc.sync.dma_start(mm_o[:], gmm[0:1, :])

            # scale = 256 / (gmax - gmin);  bias = scale * (-gmin) - 0.5
            rspan = pool.tile([128, 1], f32, tag="rspan")
            nc.vector.tensor_tensor(rspan[:], gmm[:, 0:1], gmm[:, 1:2],
                                    mybir.AluOpType.add)
            rrec = pool.tile([128, 1], f32, tag="rrec")
            nc.vector.reciprocal(rrec[:], rspan[:])
            scl = pool.tile([128, 1], f32, tag="scl")
            nc.vector.tensor_scalar_mul(scl[:], rrec[:], 256.0)
            bia = pool.tile([128, 1], f32, tag="bia")
            nc.vector.tensor_tensor(bia[:], scl[:], gmm[:, 1:2],
                                    mybir.AluOpType.mult)
            nc.vector.tensor_scalar_sub(bia[:], bia[:], 0.5)

            # ---- pass B: qb' = scale*ij + bias ; bin ; frac distance
            for p in range(PAIRS):
                if ij_tiles[p] is not None:
                    ij = ij_tiles[p]
                else:
                    ij = pdbuf.tile([128, FD], f32, tag="tld")
                    nc.sync.dma_start(ij[:], ij_spill[p][:])
                qb = pdbuf.tile([128, FD], f32, tag="a1")
                nc.scalar.activation(qb[:], ij[:],
                                     mybir.ActivationFunctionType.Identity,
                                     scale=scl[:], bias=bia[:])
                bin8 = pdbuf.tile([128, FD], mybir.dt.uint8, tag="bin8")
                nc.vector.tensor_copy(bin8[:], qb[:])
                binf = pdbuf.tile([128, FD], f32, tag="a2")
                nc.vector.tensor_copy(binf[:], bin8[:])
                d16 = pdbuf.tile([128, FD], mybir.dt.float16, tag="d16")
                nc.vector.tensor_tensor(d16[:], qb[:], binf[:],
                                        mybir.AluOpType.subtract)
                for half in range(2):
                    s = 2 * p + half
                    rows = slice(64 * half + 1, 64 * half + 63)
                    nc.sync.dma_start(
                        bins_o[s].rearrange("(w f) -> w f", w=HP),
                        bin8[rows, :])
                    nc.sync.dma_start(
                        d16_o[s].rearrange("(w f) -> w f", w=HP),
                        d16[rows, :])

    nc.finalize()
    return nc


def build_phase2(sel_rows_per_core):
    """sel_rows: list of flat row ids (b*C+c), identical program on all
    cores; each core handles one column-chunk of every selected row."""
    sel_rows = sel_rows_per_core
    n_sel = len(sel_rows)
    CHUNK = (H * W * Z) // N_CORES
    nc = bacc.Bacc("TRN2", target_bir_lowering=False, debug=False,
                   num_devices=N_CORES)
    f32 = mybir.dt.float32
    img = nc.dram_tensor("imgchunk", [B * C, CHUNK], f32,
                         kind="ExternalInput")
    out = nc.dram_tensor("sel", [n_sel, CHUNK], f32, kind="ExternalOutput")
    with tile.TileContext(nc) as tc:
        for j, row in enumerate(sel_rows):
            nc.sync.dma_start(out[j:j + 1, :], img[int(row):int(row) + 1, :])
    nc.finalize()
    return nc, n_sel


# ---------------------------------------------------------------------------
# host middle
# ---------------------------------------------------------------------------

DELTA = np.float32(2.5e-3)


def host_middle(img, k, bins_u8, d16, jnp, jax):
    """bins_u8/d16: [B*C, P_SLAB] in device (w',h',z') order.
    Returns idx [B, k] selected channel indices (descending entropy)."""
    nrows = B * C
    # base histogram from device bins
    hist = np.zeros((nrows, BINS), np.int64)
    for r in range(nrows):
        hist[r] = np.bincount(bins_u8[r], minlength=BINS)

    # flagged = samples whose qb is within DELTA of an integer boundary
    absd = np.abs(d16.astype(np.float32))
    flag = (np.float32(0.5) - absd) < DELTA
    rs, fs = np.nonzero(flag)
    # device layout flat = (w'*62 + h')*62 + z'
    wq, rem = np.divmod(fs, HP * HP)
    hq, zq = np.divmod(rem, HP)
    bq, cq = np.divmod(rs, C)

    imgf = np.asarray(img)
    # exact 27-term chain in reference order (di,dj,dk) over (h,w,z)
    s = np.zeros(len(rs), np.float32)
    for di in range(3):
        for dj in range(3):
            for dk in range(3):
                s = s + imgf[bq, cq, hq + di, wq + dj, zq + dk]
    cen = imgf[bq, cq, hq + 1, wq + 1, zq + 1]
    mean_p = (s - cen) / np.float32(26.0)
    ij_ref = cen * np.float32(100.0) + mean_p

    mn = ij_ref.min()
    mx = ij_ref.max()
    q = (ij_ref - mn) / (mx - mn)
    true_bin = np.clip(np.floor(q * np.float32(BINS)), 0, BINS - 1).astype(np.int64)

    dev_bin = bins_u8[rs, fs].astype(np.int64)
    np.subtract.at(hist, (rs, dev_bin), 1)
    np.add.at(hist, (rs, true_bin), 1)

    # entropy + topk exactly as reference (jax CPU)
    cpu = jax.devices("cpu")[0]
    with jax.default_device(cpu):
        h = jnp.asarray(hist.astype(np.float32))
        p = h / DENOM
        h_tem = -p * jnp.log(jnp.clip(p, 1e-40)) / np.float32(np.log(2.0))
        ent = h_tem.sum(axis=1).reshape(B, C)
        _, idx = jax.lax.top_k(ent, int(k))
        idx = np.asarray(idx)
    return idx, hist, (mn, mx)


def run_full(img, k, trace=False):
    import jax
    import jax.numpy as jnp
    img = np.asarray(img, dtype=np.float32)
    k = int(k)

    nc1 = build_phase1()
    band = build_band()
    imgr = img.reshape(B * C, H, W, Z)
    in_maps = [{"imgp": np.ascontiguousarray(imgr[16 * c:16 * c + 16]),
                "bandw": band} for c in range(N_CORES)]
    res1 = run_bass_kernel_spmd(nc1, in_maps, core_ids=list(range(N_CORES)),
                                trace=trace)
    bins_u8 = np.concatenate([res1.results[c]["bins"] for c in range(N_CORES)], 0)
    d16 = np.concatenate([res1.results[c]["d16"] for c in range(N_CORES)], 0)

    idx, hist, mnmx = host_middle(img, k, bins_u8, d16, jnp, jax)

    # phase 2: device gather of selected slabs, column-sharded over cores
    rows_flat = [int(b * C + ch) for b in range(B) for ch in idx[b]]
    nc2, n_sel = build_phase2(rows_flat)
    CHUNK = (H * W * Z) // N_CORES
    img2 = img.reshape(B * C, H * W * Z)
    in2 = [{"imgchunk": np.ascontiguousarray(img2[:, c * CHUNK:(c + 1) * CHUNK])}
           for c in range(N_CORES)]
    res2 = run_bass_kernel_spmd(nc2, in2, core_ids=list(range(N_CORES)),
                                trace=trace)

    out = np.zeros((B * k, H * W * Z), np.float32)
    for c in range(N_CORES):
        out[:, c * CHUNK:(c + 1) * CHUNK] = res2.results[c]["sel"]
    out = out.reshape(B, k, H, W, Z)
    return out, (res1, res2)


def kernel(**inputs):
    """Entry point: full inputs in, full output out."""
    img = np.asarray(inputs["img"], dtype=np.float32)
    k = int(np.asarray(inputs["k"]))
    out, _ = run_full(img, k)
    return out.astype(np.float32)



# revision 5
# speedup vs baseline: 3.1548x; 3.1548x over previous
"""nn_Entropy_Hist on 8 trn2 cores — single-pass device kernel.

Device (per core, 16 channel slabs): for each slab compute
v = round_u16(32*ij + 32768) where ij = k26*sum27 + (100-k26)*center over
3x3x3 valid windows. Fixed affine (no data-dependent range), so no
collective and no second pass. z-box on DVE/Pool, h-box via PE band
matmul (f32r, small k26 taps only), center term on Act in full f32,
merge + u16 cast on DVE/Pool.

Host: reconstruct ij ~ (v-32768)/32 (max dev ~1.6 v-units), find exact
global min/max from extreme candidates, bin all samples in f64, flag
samples near reference bin boundaries, recompute those exactly with the
reference f32 chain, build exact histograms -> entropy -> topk -> gather.
"""

import numpy as np

import concourse.bass as bass
import concourse.bacc as bacc
import concourse.mybir as mybir
import concourse.tile as tile
from concourse.bass_utils import run_bass_kernel_spmd

N_CORES = 8
B, C, H, W, Z = 2, 64, 64, 64, 64
HP = H - 2                      # 62
P_SLAB = HP * HP * HP           # 238328
SLABS_PER_CORE = (B * C) // N_CORES  # 16
PAIRS = SLABS_PER_CORE // 2          # 8
BINS = 256
DENOM = (H + 2) * (W + 2) * (Z + 2)

SCL = np.float32(32.0)
VBIAS = np.float32(32768.0)
K26 = np.float32(1.0) / np.float32(26.0)
C100 = np.float32(100.0) - K26          # weight of the center sample
BAND_TAP = np.float32(32.0 / 26.0)      # 32 * k26
A2_SCALE = float(SCL * C100)            # 32 * (100 - 1/26)

# host-side flag margin: max |ij_est - ij_ref| in ij units. Device error
# budget: u16 quantization 0.5/32 + DVE cast 1/32 + PE band ~0.01/32 +
# Act a2 rounding ~0.01/32  =>  ~0.049; margin below is ~1.6x that.
EPS_IJ = 0.08

FD = HP * HP                    # 3844 free elems (w', z') per partition
W_CHUNKS = [(i, min(8, HP - i)) for i in range(0, HP, 8)]


def build_band32():
    """[128,128] f32: col m sums partition rows m-1..m+1 (within each 64
    block) with weight 32/26. Output col m holds h' = m-1; cols 0,63
    per block are unused."""
    band = np.zeros((128, 128), np.float32)
    for blk in (0, 64):
        for m in range(1, 63):
            for k in (m - 1, m, m + 1):
                band[blk + k, blk + m] = BAND_TAP
    return band


def build_device():
    nc = bacc.Bacc("TRN2", target_bir_lowering=False, debug=False,
                   num_devices=N_CORES)
    f32, f32r, u16 = mybir.dt.float32, mybir.dt.float32r, mybir.dt.uint16
    imgp = nc.dram_tensor("imgp", [SLABS_PER_CORE, H, W, Z], f32,
                          kind="ExternalInput")
    bandw = nc.dram_tensor("bandw", [128, 128], f32r, kind="ExternalInput")
    v_o = nc.dram_tensor("v", [SLABS_PER_CORE, P_SLAB], u16,
                         kind="ExternalOutput")

    with tile.TileContext(nc) as tc:
        with (
            tc.tile_pool(name="pool", bufs=1) as pool,
            tc.tile_pool(name="pdbuf", bufs=2) as pdbuf,
            tc.tile_pool(name="psum", bufs=4, space="PSUM") as psum,
        ):
            band_t = pool.tile([128, 128], f32r, tag="band")
            nc.sync.dma_start(band_t[:], bandw[:])
            bias_t = pool.tile([128, 1], f32, tag="bias")
            nc.vector.memset(bias_t[:], float(VBIAS))

            for p in range(PAIRS):
                # load pair: partition = h (64 per slab), free = (w, z)
                tld = pdbuf.tile([128, W * Z], f32, tag="tld")
                tld3 = tld[:].rearrange("p (w z) -> p w z", w=W)
                for half in range(2):
                    s = 2 * p + half
                    nc.sync.dma_start(tld3[64 * half:64 * half + 64],
                                      imgp[s])

                # z-box: zb[h, w, z'] = x[z'] + x[z'+1] + x[z'+2]
                # w-split across DVE/Pool so both engines finish together
                zb = pdbuf.tile([128, W * HP], f32r, tag="zb")
                zb3 = zb[:].rearrange("p (w z) -> p w z", w=W)
                WS = 23  # DVE gets w[0:23] (plus psum merges), Pool the rest
                for eng, ws in ((nc.vector, slice(0, WS)),
                                (nc.gpsimd, slice(WS, W))):
                    eng.tensor_tensor(zb3[:, ws, :], tld3[:, ws, 0:HP],
                                      tld3[:, ws, 1:1 + HP],
                                      mybir.AluOpType.add)
                    eng.tensor_tensor(zb3[:, ws, :], zb3[:, ws, :],
                                      tld3[:, ws, 2:2 + HP],
                                      mybir.AluOpType.add)

                # center path (full f32 on Act): a2 = 32*C100*center + 32768
                a2 = pdbuf.tile([128, FD], f32, tag="a2")
                a23 = a2[:].rearrange("p (w z) -> p w z", w=HP)
                nc.scalar.activation(a23, tld3[:, 1:1 + HP, 1:1 + HP],
                                     mybir.ActivationFunctionType.Identity,
                                     scale=A2_SCALE, bias=bias_t[:])

                # per w'-chunk: 3 band matmuls (dw shifts) -> psum, then
                # merge psum + a2 -> u16 on DVE (even chunks) / Pool (odd)
                v = pdbuf.tile([128, FD], u16, tag="v")
                for ci, (w0, wn) in enumerate(W_CHUNKS):
                    ps = psum.tile([128, 8 * HP], f32, tag="ps")
                    out_ap = ps[:, 0:wn * HP]
                    for dw in range(3):
                        nc.tensor.matmul(out_ap, band_t[:],
                                         zb3[:, w0 + dw:w0 + dw + wn, :],
                                         start=(dw == 0), stop=(dw == 2))
                    sl = slice(w0 * HP, (w0 + wn) * HP)
                    # Pool cannot read PSUM; merges stay on DVE
                    nc.vector.tensor_tensor(v[:, sl], out_ap, a2[:, sl],
                                            mybir.AluOpType.add)

                for half in range(2):
                    s = 2 * p + half
                    rows = slice(64 * half + 1, 64 * half + 63)
                    nc.sync.dma_start(
                        v_o[s].rearrange("(h f) -> h f", h=HP),
                        v[rows, :])

    nc.finalize()
    return nc


# ---------------------------------------------------------------------------
# host middle
# ---------------------------------------------------------------------------

def _exact_ij(imgf, rows, hq, wq, zq):
    """Reference-exact f32 ij for samples at (row, h', w', z')."""
    bq, cq = np.divmod(rows, C)
    s = np.zeros(len(rows), np.float32)
    for di in range(3):
        for dj in range(3):
            for dk in range(3):
                s = s + imgf[bq, cq, hq + di, wq + dj, zq + dk]
    cen = imgf[bq, cq, hq + 1, wq + 1, zq + 1]
    mean_p = (s - cen) / np.float32(26.0)
    return cen * np.float32(100.0) + mean_p


def host_middle(img, k, v_u16):
    """v_u16: [B*C, P_SLAB] device codes in (h', w', z') order.
    Returns idx [B, k] (descending entropy, reference-exact)."""
    import jax
    import jax.numpy as jnp

    imgf = np.asarray(img)
    nrows = B * C
    ij_est = (v_u16.astype(np.float64) - float(VBIAS)) / float(SCL)

    def unflatten(rs, fs):
        hq, rem = np.divmod(fs, HP * HP)
        wq, zq = np.divmod(rem, HP)
        return hq, wq, zq

    # exact global min / max from extreme candidates
    est_min, est_max = ij_est.min(), ij_est.max()
    cand = np.nonzero((ij_est <= est_min + 2 * EPS_IJ) |
                      (ij_est >= est_max - 2 * EPS_IJ))
    hq, wq, zq = unflatten(*cand)
    ex = _exact_ij(imgf, cand[0], hq, wq, zq)
    mn = np.float32(ex.min())
    mx = np.float32(ex.max())

    # f64 binning of estimates against the exact f32 range
    qd = (ij_est - np.float64(mn)) * (BINS / (np.float64(mx) - np.float64(mn)))
    bins = np.clip(np.floor(qd), 0, BINS - 1).astype(np.int64)

    thr = EPS_IJ * BINS / (float(mx) - float(mn)) + 1e-3
    flag = np.abs(qd - np.rint(qd)) < thr
    frs, ffs = np.nonzero(flag)
    hq, wq, zq = unflatten(frs, ffs)
    ij_ref = _exact_ij(imgf, frs, hq, wq, zq)
    # reference-exact f32 binning for flagged samples
    q = (ij_ref - mn) / (mx - mn)
    true_bin = np.clip(np.floor(q * np.float32(BINS)), 0,
                       BINS - 1).astype(np.int64)
    bins[frs, ffs] = true_bin

    flat = (np.arange(nrows, dtype=np.int64)[:, None] * BINS + bins).ravel()
    hist = np.bincount(flat, minlength=nrows * BINS).reshape(nrows, BINS)

    cpu = jax.devices("cpu")[0]
    with jax.default_device(cpu):
        h = jnp.asarray(hist.astype(np.float32))
        p = h / DENOM
        h_tem = -p * jnp.log(jnp.clip(p, 1e-40)) / np.float32(np.log(2.0))
        ent = h_tem.sum(axis=1).reshape(B, C)
        _, idx = jax.lax.top_k(ent, int(k))
        idx = np.asarray(idx)
    return idx


def run_full(img, k, trace=False):
    img = np.asarray(img, dtype=np.float32)
    k = int(k)

    nc = build_device()
    band = build_band32()
    imgr = img.reshape(B * C, H, W, Z)
    in_maps = [{"imgp": np.ascontiguousarray(imgr[16 * c:16 * c + 16]),
                "bandw": band} for c in range(N_CORES)]
    res = run_bass_kernel_spmd(nc, in_maps, core_ids=list(range(N_CORES)),
                               trace=trace)
    v = np.concatenate([res.results[c]["v"] for c in range(N_CORES)], 0)

    idx = host_middle(img, k, v)

    out = imgr.reshape(B, C, H, W, Z)[np.arange(B)[:, None], idx]
    return np.ascontiguousarray(out), (res, v)


def kernel(**inputs):
    """Entry point: full inputs in, full output out."""
    img = np.asarray(inputs["img"], dtype=np.float32)
    k = int(np.asarray(inputs["k"]))
    out, _ = run_full(img, k)
    return out.astype(np.float32)


# revision 20
# speedup vs baseline: 5.1804x; 1.6421x over previous
"""nn_Entropy_Hist on 8 trn2 cores — single-pass device kernel.

Device (per core, 16 channel slabs): for each slab compute
v = round_u16(32*ij + 32768) where ij = k26*sum27 + (100-k26)*center over
3x3x3 valid windows. Fixed affine (no data-dependent range), so no
collective and no second pass. z-box on DVE/Pool, h-box via PE band
matmul (f32r, small k26 taps only), center term on Act in full f32,
merge + u16 cast on DVE/Pool.

Host: reconstruct ij ~ (v-32768)/32 (max dev ~1.6 v-units), find exact
global min/max from extreme candidates, bin all samples in f64, flag
samples near reference bin boundaries, recompute those exactly with the
reference f32 chain, build exact histograms -> entropy -> topk -> gather.
"""

import numpy as np

import concourse.bass as bass
import concourse.bacc as bacc
import concourse.mybir as mybir
import concourse.tile as tile
from concourse.bass_utils import run_bass_kernel_spmd

N_CORES = 8
B, C, H, W, Z = 2, 64, 64, 64, 64
HP = H - 2                      # 62
P_SLAB = HP * HP * HP           # 238328
SLABS_PER_CORE = (B * C) // N_CORES  # 16
PAIRS = SLABS_PER_CORE // 2          # 8
BINS = 256
DENOM = (H + 2) * (W + 2) * (Z + 2)

SCL = np.float32(32.0)
VBIAS = np.float32(32768.0)
K26 = np.float32(1.0) / np.float32(26.0)
C100 = np.float32(100.0) - K26          # weight of the center sample
BAND_TAP = np.float32(32.0 / 26.0)      # 32 * k26
A2_SCALE = float(SCL * C100)            # 32 * (100 - 1/26)

# host-side flag margin: max |ij_est - ij_ref| in ij units. Device error
# budget: u16 quantization 0.5/32 + DVE cast 1/32 + PE band ~0.01/32 +
# Act a2 rounding ~0.01/32  =>  ~0.049; margin below is ~1.6x that.
EPS_IJ = 0.08

FD = HP * HP                    # 3844 free elems (w', z') per partition
W_CHUNKS = [(i, min(8, HP - i)) for i in range(0, HP, 8)]


def build_band32():
    """[128,128] f32: col m sums partition rows m-1..m+1 (within each 64
    block) with weight 32/26. Output col m holds h' = m-1; cols 0,63
    per block are unused."""
    band = np.zeros((128, 128), np.float32)
    for blk in (0, 64):
        for m in range(1, 63):
            for k in (m - 1, m, m + 1):
                band[blk + k, blk + m] = BAND_TAP
    return band


def build_device(ws=31, bufs_tld=4, bufs_zb=3, bufs_a2=3, bufs_kb=2,
                 bufs_v=2, bufs_ps=4, fused_load=True, merge_eng="dve",
                 merge_splits=2, evac=False, ahead=2, late_a2=False,
                 psum_group=2, zsub=1):
    """ws: w-split of z-box between DVE [0:ws] and Pool [ws:64].
    evac: True -> Act evacuates psum to kb, big merges from kb;
          False -> DVE merges straight from psum per chunk.
    merge_eng: 'dve'|'pool' (pool only valid with evac).
    ahead: how many pairs ahead prep() runs."""
    nc = bacc.Bacc("TRN2", target_bir_lowering=False, debug=False,
                   num_devices=N_CORES)
    f32, f32r, u16 = mybir.dt.float32, mybir.dt.float32r, mybir.dt.uint16
    imgp = nc.dram_tensor("imgp", [SLABS_PER_CORE, H, W, Z], f32,
                          kind="ExternalInput")
    bandw = nc.dram_tensor("bandw", [128, 128], f32r, kind="ExternalInput")
    v_o = nc.dram_tensor("v", [SLABS_PER_CORE, P_SLAB], u16,
                         kind="ExternalOutput")

    with tile.TileContext(nc) as tc:
        with (
            tc.tile_pool(name="pool", bufs=1) as pool,
            tc.tile_pool(name="ptld", bufs=bufs_tld) as ptld,
            tc.tile_pool(name="pzb", bufs=bufs_zb) as pzb,
            tc.tile_pool(name="pa2", bufs=bufs_a2) as pa2,
            tc.tile_pool(name="pkb", bufs=bufs_kb) as pkb,
            tc.tile_pool(name="pv", bufs=bufs_v) as pv,
            tc.tile_pool(name="psum", bufs=bufs_ps, space="PSUM") as psum,
        ):
            band_t = pool.tile([128, 128], f32r, tag="band")
            nc.sync.dma_start(band_t[:], bandw[:])
            bias_t = pool.tile([128, 1], f32, tag="bias")
            nc.vector.memset(bias_t[:], float(VBIAS))

            tlds = [None] * PAIRS
            zbs = [None] * PAIRS
            a2s = [None] * PAIRS

            def load(p):
                tld = ptld.tile([128, W * Z], f32, tag="tld")
                tld3 = tld[:].rearrange("p (w z) -> p w z", w=W)
                if fused_load:
                    src = imgp[2 * p:2 * p + 2].rearrange(
                        "s h w z -> (s h) w z")
                    nc.sync.dma_start(tld3[:], src)
                else:
                    for half in range(2):
                        nc.sync.dma_start(tld3[64 * half:64 * half + 64],
                                          imgp[2 * p + half])
                tlds[p] = tld3

            def prep(p):
                """z-box (DVE/Pool split) + center affine (Act) for pair p."""
                tld3 = tlds[p]
                zb = pzb.tile([128, W * HP], f32r, tag="zb")
                zb3 = zb[:].rearrange("p (w z) -> p w z", w=W)
                engs = []
                if ws > 0:
                    engs.append((nc.vector, 0, ws))
                if ws < W:
                    engs.append((nc.gpsimd, ws, W))
                for eng, lo, hi in engs:
                    bounds = [lo + (hi - lo) * i // zsub for i in range(zsub)]
                    bounds.append(hi)
                    for si in range(zsub):
                        wsl = slice(bounds[si], bounds[si + 1])
                        eng.tensor_tensor(zb3[:, wsl, :], tld3[:, wsl, 0:HP],
                                          tld3[:, wsl, 1:1 + HP],
                                          mybir.AluOpType.add)
                        eng.tensor_tensor(zb3[:, wsl, :], zb3[:, wsl, :],
                                          tld3[:, wsl, 2:2 + HP],
                                          mybir.AluOpType.add)
                if not late_a2:
                    mk_a2(p)
                zbs[p] = zb3

            def mk_a2(p):
                tld3 = tlds[p]
                a2 = pa2.tile([128, FD], f32, tag="a2")
                a23 = a2[:].rearrange("p (w z) -> p w z", w=HP)
                nc.scalar.activation(a23, tld3[:, 1:1 + HP, 1:1 + HP],
                                     mybir.ActivationFunctionType.Identity,
                                     scale=A2_SCALE, bias=bias_t[:])
                a2s[p] = a2

            def compute(p):
                if late_a2:
                    mk_a2(p)
                zb3, a2 = zbs[p], a2s[p]
                v = pv.tile([128, FD], u16, tag="v")
                # psum packing: psum_group chunks per psum tile (512-f32
                # aligned) so DVE merges groups of chunks in one op.
                if psum_group == 4:
                    groups = [(0, 4), (4, 7), (7, 8)]
                elif psum_group == 2:
                    groups = [(0, 2), (2, 4), (4, 6), (6, 7), (7, 8)]
                else:
                    groups = [(i, i + 1) for i in range(8)]
                for g0, g1 in groups:
                    ng = g1 - g0
                    ps = psum.tile([128, 512 * psum_group], f32, tag="ps")
                    for j, ci in enumerate(range(g0, g1)):
                        w0, wn = W_CHUNKS[ci]
                        out_ap = ps[:, 512 * j:512 * j + wn * HP]
                        for dw in range(3):
                            nc.tensor.matmul(out_ap, band_t[:],
                                             zb3[:, w0 + dw:w0 + dw + wn, :],
                                             start=(dw == 0), stop=(dw == 2))
                    c0 = W_CHUNKS[g0][0] * HP
                    we = W_CHUNKS[g1 - 1]
                    c1 = (we[0] + we[1]) * HP
                    span = c1 - c0  # ng * wn*HP, uniform within group
                    wcols = span // ng
                    psv = ps[:].rearrange("p (b c) -> p b c", b=psum_group)
                    a2v = a2[:, c0:c1].rearrange("p (b c) -> p b c", b=ng)
                    vv = v[:, c0:c1].rearrange("p (b c) -> p b c", b=ng)
                    nc.vector.tensor_tensor(vv, psv[:, 0:ng, 0:wcols], a2v,
                                            mybir.AluOpType.add)
                for hsl in (slice(0, 1984), slice(1984, FD)):
                    for half in range(2):
                        s = 2 * p + half
                        rows = slice(64 * half + 1, 64 * half + 63)
                        dst = v_o[s].rearrange("(h f) -> h f", h=HP)
                        nc.sync.dma_start(dst[:, hsl], v[rows, hsl])
                tlds[p] = zbs[p] = a2s[p] = None

            for p in range(min(ahead + 1, PAIRS)):
                load(p)
            for p in range(min(ahead, PAIRS)):
                prep(p)
            for p in range(PAIRS):
                if p + ahead + 1 < PAIRS:
                    load(p + ahead + 1)
                if p + ahead < PAIRS:
                    prep(p + ahead)
                compute(p)

    nc.finalize()
    return nc


# ---------------------------------------------------------------------------
# host middle
# ---------------------------------------------------------------------------

def _exact_ij(imgf, rows, hq, wq, zq):
    """Reference-exact f32 ij for samples at (row, h', w', z')."""
    bq, cq = np.divmod(rows, C)
    s = np.zeros(len(rows), np.float32)
    for di in range(3):
        for dj in range(3):
            for dk in range(3):
                s = s + imgf[bq, cq, hq + di, wq + dj, zq + dk]
    cen = imgf[bq, cq, hq + 1, wq + 1, zq + 1]
    mean_p = (s - cen) / np.float32(26.0)
    return cen * np.float32(100.0) + mean_p


def host_middle(img, k, v_u16):
    """v_u16: [B*C, P_SLAB] device codes in (h', w', z') order.
    Returns idx [B, k] (descending entropy, reference-exact)."""
    import jax
    import jax.numpy as jnp

    imgf = np.asarray(img)
    nrows = B * C
    ij_est = (v_u16.astype(np.float64) - float(VBIAS)) / float(SCL)

    def unflatten(rs, fs):
        hq, rem = np.divmod(fs, HP * HP)
        wq, zq = np.divmod(rem, HP)
        return hq, wq, zq

    # exact global min / max from extreme candidates
    est_min, est_max = ij_est.min(), ij_est.max()
    cand = np.nonzero((ij_est <= est_min + 2 * EPS_IJ) |
                      (ij_est >= est_max - 2 * EPS_IJ))
    hq, wq, zq = unflatten(*cand)
    ex = _exact_ij(imgf, cand[0], hq, wq, zq)
    mn = np.float32(ex.min())
    mx = np.float32(ex.max())

    # f64 binning of estimates against the exact f32 range
    qd = (ij_est - np.float64(mn)) * (BINS / (np.float64(mx) - np.float64(mn)))
    bins = np.clip(np.floor(qd), 0, BINS - 1).astype(np.int64)

    thr = EPS_IJ * BINS / (float(mx) - float(mn)) + 1e-3
    flag = np.abs(qd - np.rint(qd)) < thr
    frs, ffs = np.nonzero(flag)
    hq, wq, zq = unflatten(frs, ffs)
    ij_ref = _exact_ij(imgf, frs, hq, wq, zq)
    # reference-exact f32 binning for flagged samples
    q = (ij_ref - mn) / (mx - mn)
    true_bin = np.clip(np.floor(q * np.float32(BINS)), 0,
                       BINS - 1).astype(np.int64)
    bins[frs, ffs] = true_bin

    flat = (np.arange(nrows, dtype=np.int64)[:, None] * BINS + bins).ravel()
    hist = np.bincount(flat, minlength=nrows * BINS).reshape(nrows, BINS)

    cpu = jax.devices("cpu")[0]
    with jax.default_device(cpu):
        h = jnp.asarray(hist.astype(np.float32))
        p = h / DENOM
        h_tem = -p * jnp.log(jnp.clip(p, 1e-40)) / np.float32(np.log(2.0))
        ent = h_tem.sum(axis=1).reshape(B, C)
        _, idx = jax.lax.top_k(ent, int(k))
        idx = np.asarray(idx)
    return idx


def run_full(img, k, trace=False):
    img = np.asarray(img, dtype=np.float32)
    k = int(k)

    nc = build_device()
    band = build_band32()
    imgr = img.reshape(B * C, H, W, Z)
    in_maps = [{"imgp": np.ascontiguousarray(imgr[16 * c:16 * c + 16]),
                "bandw": band} for c in range(N_CORES)]
    res = run_bass_kernel_spmd(nc, in_maps, core_ids=list(range(N_CORES)),
                               trace=trace)
    v = np.concatenate([res.results[c]["v"] for c in range(N_CORES)], 0)

    idx = host_middle(img, k, v)

    out = imgr.reshape(B, C, H, W, Z)[np.arange(B)[:, None], idx]
    return np.ascontiguousarray(out), (res, v)


def kernel(**inputs):
    """Entry point: full inputs in, full output out."""
    img = np.asarray(inputs["img"], dtype=np.float32)
    k = int(np.asarray(inputs["k"]))
    out, _ = run_full(img, k)
    return out.astype(np.float32)
